# revision 17
# baseline (speedup 1.0000x reference)
"""Causal self-attention (S=2048, D=1024, 16 heads x 64) on 8 Trainium2 cores.

Tensor-parallel sharding: 2 heads per core. Each core computes
  qkv_local = x @ Wqkv[:, local]      (local q/k/v columns, q pre-scaled 1/8)
  attn_h    = softmax(mask(q_h k_h^T)) v_h          for its 2 heads
  partial   = concat(attn) @ Wout[local_rows, :]    (128 rows of Wout)
and the host sums the 8 bf16 partials (+bias).

All on-chip data is bf16 (PSUM accumulation in fp32); end-to-end rel err
vs the fp32 reference is ~4.4e-3, dominated by the input rounding.

On-chip layout: q^T/k^T are single [128, S] tiles with head-dim on
partitions (h0 rows 0-63, h1 rows 64-127); logit matmuls contract K=64
over the head's partition range. Logits are computed transposed
([key, query]) so exp(logits) blocks feed the probs@v matmul directly as
the moving operand; v carries a ones-column per head so the same
accumulation also produces the softmax row-sums. No max-subtraction:
logits are ~N(0,1) after the 1/8 scale. Masked entries are zeroed after
exp (GpSimd, off the PE/ACT critical path).

Schedule notes: 8 warm-up matmuls on a zero tile run during the initial
DMA wait so the PE_HAM clock gate (1.2 -> 2.4 GHz after ~3.4us of
sustained activity) is already open when real work starts. Attention
emits both heads' logit groups interleaved so one head's PE matmuls hide
the other head's serial exp latency on the scalar engine.
"""

import numpy as np

import concourse.bass as bass
import concourse.mybir as mybir
import concourse.tile as tile
from concourse import bacc
from concourse.bass_utils import run_bass_kernel_spmd

S = 2048
D = 1024
DH = 64
N_CORES = 8

P = 128
NB512 = S // 512  # 512-wide query chunks
NB128 = S // 128  # 128-wide chunks
KO = D // P  # contraction chunks for the projections

F32 = mybir.dt.float32

_compiled = {}


def _emit(nc, tc, mm_dt, xt, w, wout, maskt, ident, out):
    f32 = F32
    with (
        tc.tile_pool(name="const", bufs=1) as const,
        tc.tile_pool(name="epool", bufs=8) as epool,
        tc.tile_pool(name="opool", bufs=6) as opool,
        tc.tile_pool(name="rcpool", bufs=1) as rcpool,
        tc.tile_pool(name="psmm", bufs=2, space="PSUM") as psmm,
        tc.tile_pool(name="psacc", bufs=2, space="PSUM") as psacc,
    ):
        sb_xT = const.tile([P, KO, S], mm_dt, name="sb_xT")
        sb_w = const.tile([P, KO, 384], mm_dt, name="sb_w")
        sb_wout = const.tile([P, D], mm_dt, name="sb_wout")
        sb_mask = const.tile([P, 4, 512], mm_dt, name="sb_mask")
        # per-head q^T/k^T padded to K=128 with zero rows 64-127: K=64
        # matmuls run slow and do not register PE activity for the HAM
        # clock gate (observed: throttle_active jumps 18us -> 80us)
        sb_qT = [const.tile([P, S], mm_dt, name=f"sb_qT{h}") for h in (0, 1)]
        sb_kT = [const.tile([P, S], mm_dt, name=f"sb_kT{h}") for h in (0, 1)]
        sb_v = const.tile([P, NB128, 130], mm_dt, name="sb_v")
        sb_vT = const.tile([P, S], mm_dt, name="sb_vT")
        sb_attnT = const.tile([P, S], mm_dt, name="sb_attnT")
        sb_ident = const.tile([P, P], mm_dt, name="sb_ident")
        sb_warm = const.tile([P, 512], mm_dt, name="sb_warm")

        # PE warm-up: ~3.4us of dependency-free matmuls on a zero tile keep
        # the HAM activity window busy while the first input DMAs stream, so
        # the clock gate is fully open (2.4 GHz) when real matmuls start.
        # Tapered sizes: once the first real operands land, at most one
        # small spam matmul delays the real stream.
        nc.vector.memset(sb_warm[:], 0.0)
        for rows in [512] * 6 + [128] * 10:
            wps = psmm.tile([P, 512], f32, name="ps_warm", tag="mm")
            nc.tensor.matmul(
                wps[:, :rows], sb_warm[:, 0:P], sb_warm[:, :rows],
                start=True, stop=True,
            )

        # loads: small weights on the GpSimd SWDGE queue; xT column blocks
        # si-major so the first q/k chunk only waits on the first ~1MB
        for o in range(KO):
            weng = nc.scalar if o % 2 == 0 else nc.gpsimd
            weng.dma_start(sb_w[:, o, :], w[o * P : (o + 1) * P, :])
        xt3 = xt.rearrange("(o p) s -> p o s", p=P)
        for si in range(NB512):
            sl = slice(si * 512, (si + 1) * 512)
            for oo in range(0, KO, 2):
                nc.sync.dma_start(
                    sb_xT[:, oo : oo + 2, sl], xt3[:, oo : oo + 2, sl]
                )
        nc.gpsimd.dma_start(sb_mask[:], maskt[:])
        nc.gpsimd.dma_start(sb_wout[:], wout[:])
        nc.gpsimd.dma_start(sb_ident[:], ident[:])
        nc.gpsimd.memset(sb_v[:, :, DH], 1.0)
        nc.gpsimd.memset(sb_v[:, :, 129], 1.0)
        for h in (0, 1):
            nc.gpsimd.memset(sb_qT[h][DH:P, :], 0.0)
            nc.gpsimd.memset(sb_kT[h][DH:P, :], 0.0)

        # q^T / k^T producer: [c, s] = sum_D W[D, c] * xT[D, s]
        def emit_qk(si):
            for cc, dest in ((0, sb_qT), (1, sb_kT)):
                ps = psmm.tile([P, 512], f32, name="ps_qk", tag="mm")
                for o in range(KO):
                    nc.tensor.matmul(
                        ps[:],
                        sb_w[:, o, cc * P : (cc + 1) * P],
                        sb_xT[:, o, si * 512 : (si + 1) * 512],
                        start=(o == 0),
                        stop=(o == KO - 1),
                    )
                sl = slice(si * 512, (si + 1) * 512)
                nc.vector.tensor_copy(dest[0][0:DH, sl], ps[0:DH, :])
                nc.vector.tensor_copy(dest[1][0:DH, sl], ps[DH:P, :])

        # v^T producer (same efficient N=512 shape as q/k), then PE-mode
        # transposes turn each 128x128 block into v natural layout
        def emit_vT(si):
            psv = psmm.tile([P, 512], f32, name="ps_vT", tag="mm")
            for o in range(KO):
                nc.tensor.matmul(
                    psv[:],
                    sb_w[:, o, 256:384],
                    sb_xT[:, o, si * 512 : (si + 1) * 512],
                    start=(o == 0),
                    stop=(o == KO - 1),
                )
            nc.vector.tensor_copy(sb_vT[:, si * 512 : (si + 1) * 512], psv[:])

        def emit_v(sc):
            pt = psmm.tile([P, P], mm_dt, name="ps_t", tag="mm")
            nc.tensor.transpose(
                pt[:], sb_vT[:, sc * P : (sc + 1) * P], sb_ident[:]
            )
            nc.vector.tensor_copy(sb_v[:, sc, 0:DH], pt[:, 0:DH])
            nc.vector.tensor_copy(sb_v[:, sc, DH + 1 : 129], pt[:, DH:P])

        # output projection for one 128-row query chunk
        def emit_proj(sc):
            for ec in range(D // 512):
                pp = psmm.tile([P, 512], f32, name="ps_p", tag="mm")
                nc.tensor.matmul(
                    pp[:],
                    sb_attnT[:, sc * P : (sc + 1) * P],
                    sb_wout[:, ec * 512 : (ec + 1) * 512],
                    start=True,
                    stop=True,
                )
                ot = opool.tile([P, 512], mm_dt, name="ot", tag="ot")
                nc.vector.tensor_copy(ot[:], pp[:])
                nc.sync.dma_start(
                    out[sc * P : (sc + 1) * P, ec * 512 : (ec + 1) * 512], ot[:]
                )

        # interleaved schedule: produce q/k/v just-in-time, run causal
        # attention blocks (logits kept transposed [j, i]), project each
        # query chunk as soon as both heads are normalized
        emit_qk(0)
        emit_vT(0)
        for sc in range(4):
            emit_v(sc)
        for ic in range(NB512):
            if ic + 1 < NB512:
                emit_qk(ic + 1)
                emit_vT(ic + 1)
                for sc in range(4 * (ic + 1), 4 * (ic + 2)):
                    emit_v(sc)
            for h in (0, 1):
                po = h * DH
                njc = 4 * (ic + 1)
                # pair logit blocks into shared 2-bank PSUM tiles (one exp
                # covers each pair); diagonal blocks are narrowed to the
                # columns that survive the causal mask: jc=4ic+r only needs
                # query columns i >= 128r within the 512-block
                groups = []  # list of [(jc, col_start, n, i0), ...]
                for jp in range(2 * ic):
                    groups.append(
                        [(2 * jp, 0, 512, 0), (2 * jp + 1, 512, 512, 0)]
                    )
                groups.append(
                    [(4 * ic, 0, 512, 0), (4 * ic + 1, 512, 384, 128)]
                )
                groups.append(
                    [(4 * ic + 2, 0, 256, 256), (4 * ic + 3, 256, 256, 256)]
                )
                es = {}
                for grp in groups:
                    tot = grp[-1][1] + grp[-1][2]
                    pl = psmm.tile([P, 1024], f32, name="ps_l", tag="mm2")
                    for jc, cs, n, i0 in grp:
                        nc.tensor.matmul(
                            pl[:, cs : cs + n],
                            sb_kT[h][:, jc * P : (jc + 1) * P],
                            sb_qT[h][:, ic * 512 + i0 : ic * 512 + i0 + n],
                            start=True,
                            stop=True,
                        )
                    e = epool.tile([P, 1024], mm_dt, name="e_t", tag="e")
                    nc.scalar.activation(
                        e[:, :tot], pl[:, :tot], mybir.ActivationFunctionType.Exp
                    )
                    for jc, cs, n, i0 in grp:
                        r = jc - 4 * ic
                        eh = e[:, cs : cs + n]
                        if r >= 0:
                            nc.gpsimd.tensor_mul(
                                eh, eh, sb_mask[:, r, i0 : i0 + n]
                            )
                        es[jc] = (eh, i0, n)
                acc = psacc.tile([DH + 1, 512], f32, name="ps_acc", tag="acc")
                for jc in range(njc):
                    eh, i0, n = es[jc]
                    nc.tensor.matmul(
                        acc[:, i0 : i0 + n],
                        sb_v[:, jc, h * 65 : (h + 1) * 65],
                        eh,
                        start=(jc == 0),
                        stop=(jc == njc - 1),
                    )
                # normalize: reciprocal of the rowsum row, broadcast across
                # partitions on the (otherwise idle) GpSimd engine, then one
                # PSUM-reading multiply straight into attnT
                rsk = rcpool.tile([1, 512], f32, name="rsk", tag="rsk", bufs=2)
                nc.scalar.copy(rsk[:], acc[DH : DH + 1, :])
                rck = rcpool.tile([1, 512], f32, name="rck", tag="rck", bufs=3)
                nc.vector.reciprocal_approx_fast(rck[:], rsk[:])
                bck = rcpool.tile([DH, 512], f32, name="bck", tag="bck", bufs=3)
                nc.gpsimd.partition_broadcast(bck[:], rck[:])
                dst = sb_attnT[po : po + DH, ic * 512 : (ic + 1) * 512]
                nc.vector.tensor_mul(dst, acc[0:DH, :], bck[:])
            # project this query block right away to overlap the output DMA
            # with the remaining attention blocks
            for sc in range(4 * ic, 4 * (ic + 1)):
                emit_proj(sc)


def build(mm_dt=mybir.dt.bfloat16):
    key = str(mm_dt)
    if key in _compiled:
        return _compiled[key]
    nc = bacc.Bacc("TRN2", target_bir_lowering=False, debug=False, num_devices=N_CORES)
    xt = nc.dram_tensor("xt", [D, S], mm_dt, kind="ExternalInput").ap()
    w = nc.dram_tensor("w", [D, 384], mm_dt, kind="ExternalInput").ap()
    wout = nc.dram_tensor("wout", [P, D], mm_dt, kind="ExternalInput").ap()
    maskt = nc.dram_tensor("maskt", [P, 4, 512], mm_dt, kind="ExternalInput").ap()
    ident = nc.dram_tensor("ident", [P, P], mm_dt, kind="ExternalInput").ap()
    out = nc.dram_tensor("out", [S, D], mm_dt, kind="ExternalOutput").ap()
    with tile.TileContext(nc) as tc:
        _emit(nc, tc, mm_dt, xt, w, wout, maskt, ident, out)
    nc.compile()
    _compiled[key] = nc
    return nc


def _np_dt(mm_dt):
    if mm_dt == mybir.dt.bfloat16:
        import ml_dtypes

        return ml_dtypes.bfloat16
    return np.float32


def make_inputs(x, Wqkv, Wout, np_dt):
    """Host-side shard/layout prep -> per-core input maps."""
    x = np.ascontiguousarray(np.asarray(x, np.float32))
    Wqkv = np.asarray(Wqkv, np.float32)
    Wout = np.asarray(Wout, np.float32)
    xT = np.ascontiguousarray(x.T).astype(np_dt)  # [D, S]
    j = np.arange(512, dtype=np.int64)
    m512 = (j[:, None] <= j[None, :]).astype(np.float32)  # [J, i]: J <= i
    mask = np.ascontiguousarray(
        m512.reshape(4, 128, 512).transpose(1, 0, 2)
    ).astype(np_dt)  # [p, r, i] = (128r + p <= i)
    in_maps = []
    for c in range(N_CORES):
        wq = Wqkv[:, 128 * c : 128 * (c + 1)] * (1.0 / np.sqrt(DH))
        wk = Wqkv[:, D + 128 * c : D + 128 * (c + 1)]
        wv = Wqkv[:, 2 * D + 128 * c : 2 * D + 128 * (c + 1)]
        w_loc = np.ascontiguousarray(np.concatenate([wq, wk, wv], axis=1))
        wout_loc = np.ascontiguousarray(Wout[128 * c : 128 * (c + 1), :])
        in_maps.append(
            {
                "xt": xT,
                "w": w_loc.astype(np_dt),
                "wout": wout_loc.astype(np_dt),
                "maskt": mask,
                "ident": np.eye(P, dtype=np_dt),
            }
        )
    return in_maps


def kernel(x, Wqkv, Wout, bias, mm_dt=mybir.dt.bfloat16, **run_kwargs):
    nc = build(mm_dt)
    in_maps = make_inputs(x, Wqkv, Wout, _np_dt(mm_dt))
    res = run_bass_kernel_spmd(nc, in_maps, core_ids=list(range(N_CORES)), **run_kwargs)
    acc = np.zeros((S, D), np.float64)
    for c in range(N_CORES):
        acc += res.results[c]["out"].astype(np.float64)
    acc += np.asarray(bias, np.float64)[None, :]
    return acc.astype(np.float32)


# revision 18
# speedup vs baseline: 1.7772x; 1.7772x over previous
"""Causal self-attention (S=2048, D=1024, 16 heads x 64) on 8 Trainium2 cores.

Tensor-parallel sharding: 2 heads per core. Each core computes
  qkv_local = x @ Wqkv[:, local]      (local q/k/v columns, q pre-scaled 1/8)
  attn_h    = softmax(mask(q_h k_h^T)) v_h          for its 2 heads
  partial   = concat(attn) @ Wout[local_rows, :]    (128 rows of Wout)
and the host sums the 8 bf16 partials (+bias).

All on-chip data is bf16 (PSUM accumulation in fp32); end-to-end rel err
vs the fp32 reference is ~4.4e-3, dominated by the input rounding.

On-chip layout: q^T/k^T are single [128, S] tiles with head-dim on
partitions (h0 rows 0-63, h1 rows 64-127); logit matmuls contract K=64
over the head's partition range. Logits are computed transposed
([key, query]) so exp(logits) blocks feed the probs@v matmul directly as
the moving operand; v carries a ones-column per head so the same
accumulation also produces the softmax row-sums. No max-subtraction:
logits are ~N(0,1) after the 1/8 scale. Masked entries are zeroed after
exp (GpSimd, off the PE/ACT critical path).

Schedule notes: 8 warm-up matmuls on a zero tile run during the initial
DMA wait so the PE_HAM clock gate (1.2 -> 2.4 GHz after ~3.4us of
sustained activity) is already open when real work starts. Attention
emits both heads' logit groups interleaved so one head's PE matmuls hide
the other head's serial exp latency on the scalar engine.
"""

import numpy as np

import concourse.bass as bass
import concourse.mybir as mybir
import concourse.tile as tile
from concourse import bacc
from concourse.bass_utils import run_bass_kernel_spmd

S = 2048
D = 1024
DH = 64
N_CORES = 8

P = 128
NB512 = S // 512  # 512-wide query chunks
NB128 = S // 128  # 128-wide chunks
KO = D // P  # contraction chunks for the projections

F32 = mybir.dt.float32

_compiled = {}


def _emit(nc, tc, mm_dt, xt, w, wout, maskt, ident, out):
    f32 = F32
    with (
        tc.tile_pool(name="const", bufs=1) as const,
        tc.tile_pool(name="epool", bufs=8) as epool,
        tc.tile_pool(name="opool", bufs=6) as opool,
        tc.tile_pool(name="rcpool", bufs=1) as rcpool,
        tc.tile_pool(name="psmm", bufs=2, space="PSUM") as psmm,
        tc.tile_pool(name="psacc", bufs=2, space="PSUM") as psacc,
    ):
        sb_xT = const.tile([P, KO, S], mm_dt, name="sb_xT")
        sb_w = const.tile([P, KO, 384], mm_dt, name="sb_w")
        sb_wout = const.tile([P, D], mm_dt, name="sb_wout")
        sb_mask = const.tile([P, 4, 512], mm_dt, name="sb_mask")
        # per-head q^T/k^T padded to K=128 with zero rows 64-127: K=64
        # matmuls run slow and do not register PE activity for the HAM
        # clock gate (observed: throttle_active jumps 18us -> 80us)
        sb_qT = [const.tile([P, S], mm_dt, name=f"sb_qT{h}") for h in (0, 1)]
        sb_kT = [const.tile([P, S], mm_dt, name=f"sb_kT{h}") for h in (0, 1)]
        sb_v = const.tile([P, NB128, 130], mm_dt, name="sb_v")
        sb_vT = const.tile([P, S], mm_dt, name="sb_vT")
        sb_attnT = const.tile([P, S], mm_dt, name="sb_attnT")
        sb_ident = const.tile([P, P], mm_dt, name="sb_ident")
        sb_warm = const.tile([P, 512], mm_dt, name="sb_warm")

        # PE warm-up: ~3.4us of dependency-free matmuls on a zero tile keep
        # the HAM activity window busy while the first input DMAs stream, so
        # the clock gate is fully open (2.4 GHz) when real matmuls start.
        # Tapered sizes: once the first real operands land, at most one
        # small spam matmul delays the real stream.
        nc.vector.memset(sb_warm[:], 0.0)
        for rows in [512] * 6 + [128] * 10:
            wps = psmm.tile([P, 512], f32, name="ps_warm", tag="mm")
            nc.tensor.matmul(
                wps[:, :rows], sb_warm[:, 0:P], sb_warm[:, :rows],
                start=True, stop=True,
            )

        # loads: small weights on the GpSimd SWDGE queue; xT column blocks
        # si-major so the first q/k chunk only waits on the first ~1MB
        for o in range(KO):
            weng = nc.scalar if o % 2 == 0 else nc.gpsimd
            weng.dma_start(sb_w[:, o, :], w[o * P : (o + 1) * P, :])
        xt3 = xt.rearrange("(o p) s -> p o s", p=P)
        for si in range(NB512):
            sl = slice(si * 512, (si + 1) * 512)
            for oo in range(0, KO, 2):
                nc.sync.dma_start(
                    sb_xT[:, oo : oo + 2, sl], xt3[:, oo : oo + 2, sl]
                )
        nc.gpsimd.dma_start(sb_mask[:], maskt[:])
        nc.gpsimd.dma_start(sb_wout[:], wout[:])
        nc.gpsimd.dma_start(sb_ident[:], ident[:])
        nc.gpsimd.memset(sb_v[:, :, DH], 1.0)
        nc.gpsimd.memset(sb_v[:, :, 129], 1.0)
        for h in (0, 1):
            nc.gpsimd.memset(sb_qT[h][DH:P, :], 0.0)
            nc.gpsimd.memset(sb_kT[h][DH:P, :], 0.0)

        # q^T / k^T producer: [c, s] = sum_D W[D, c] * xT[D, s]
        def emit_qk(si):
            for cc, dest in ((0, sb_qT), (1, sb_kT)):
                ps = psmm.tile([P, 512], f32, name="ps_qk", tag="mm")
                for o in range(KO):
                    nc.tensor.matmul(
                        ps[:],
                        sb_w[:, o, cc * P : (cc + 1) * P],
                        sb_xT[:, o, si * 512 : (si + 1) * 512],
                        start=(o == 0),
                        stop=(o == KO - 1),
                    )
                sl = slice(si * 512, (si + 1) * 512)
                nc.vector.tensor_copy(dest[0][0:DH, sl], ps[0:DH, :])
                nc.vector.tensor_copy(dest[1][0:DH, sl], ps[DH:P, :])

        # v^T producer (same efficient N=512 shape as q/k), then PE-mode
        # transposes turn each 128x128 block into v natural layout
        def emit_vT(si):
            psv = psmm.tile([P, 512], f32, name="ps_vT", tag="mm")
            for o in range(KO):
                nc.tensor.matmul(
                    psv[:],
                    sb_w[:, o, 256:384],
                    sb_xT[:, o, si * 512 : (si + 1) * 512],
                    start=(o == 0),
                    stop=(o == KO - 1),
                )
            nc.vector.tensor_copy(sb_vT[:, si * 512 : (si + 1) * 512], psv[:])

        def emit_v(sc):
            pt = psmm.tile([P, P], mm_dt, name="ps_t", tag="mm")
            nc.tensor.transpose(
                pt[:], sb_vT[:, sc * P : (sc + 1) * P], sb_ident[:]
            )
            nc.vector.tensor_copy(sb_v[:, sc, 0:DH], pt[:, 0:DH])
            nc.vector.tensor_copy(sb_v[:, sc, DH + 1 : 129], pt[:, DH:P])

        # output projection for one 128-row query chunk
        def emit_proj(sc):
            for ec in range(D // 512):
                pp = psmm.tile([P, 512], f32, name="ps_p", tag="mm")
                nc.tensor.matmul(
                    pp[:],
                    sb_attnT[:, sc * P : (sc + 1) * P],
                    sb_wout[:, ec * 512 : (ec + 1) * 512],
                    start=True,
                    stop=True,
                )
                ot = opool.tile([P, 512], mm_dt, name="ot", tag="ot")
                nc.vector.tensor_copy(ot[:], pp[:])
                nc.sync.dma_start(
                    out[sc * P : (sc + 1) * P, ec * 512 : (ec + 1) * 512], ot[:]
                )

        # interleaved schedule: produce q/k/v just-in-time, run causal
        # attention blocks (logits kept transposed [j, i]), project each
        # query chunk as soon as both heads are normalized
        emit_qk(0)
        emit_vT(0)
        for sc in range(4):
            emit_v(sc)
        for ic in range(NB512):
            if ic + 1 < NB512:
                emit_qk(ic + 1)
                emit_vT(ic + 1)
                for sc in range(4 * (ic + 1), 4 * (ic + 2)):
                    emit_v(sc)
            for h in (0, 1):
                po = h * DH
                njc = 4 * (ic + 1)
                # pair logit blocks into shared 2-bank PSUM tiles (one exp
                # covers each pair); diagonal blocks are narrowed to the
                # columns that survive the causal mask: jc=4ic+r only needs
                # query columns i >= 128r within the 512-block
                groups = []  # list of [(jc, col_start, n, i0), ...]
                for jp in range(2 * ic):
                    groups.append(
                        [(2 * jp, 0, 512, 0), (2 * jp + 1, 512, 512, 0)]
                    )
                groups.append(
                    [(4 * ic, 0, 512, 0), (4 * ic + 1, 512, 384, 128)]
                )
                groups.append(
                    [(4 * ic + 2, 0, 256, 256), (4 * ic + 3, 256, 256, 256)]
                )
                es = {}
                for grp in groups:
                    tot = grp[-1][1] + grp[-1][2]
                    pl = psmm.tile([P, 1024], f32, name="ps_l", tag="mm2")
                    for jc, cs, n, i0 in grp:
                        nc.tensor.matmul(
                            pl[:, cs : cs + n],
                            sb_kT[h][:, jc * P : (jc + 1) * P],
                            sb_qT[h][:, ic * 512 + i0 : ic * 512 + i0 + n],
                            start=True,
                            stop=True,
                        )
                    e = epool.tile([P, 1024], mm_dt, name="e_t", tag="e")
                    nc.scalar.activation(
                        e[:, :tot], pl[:, :tot], mybir.ActivationFunctionType.Exp
                    )
                    for jc, cs, n, i0 in grp:
                        r = jc - 4 * ic
                        eh = e[:, cs : cs + n]
                        if r >= 0:
                            nc.vector.tensor_mul(
                                eh, eh, sb_mask[:, r, i0 : i0 + n]
                            )
                        es[jc] = (eh, i0, n)
                acc = psacc.tile([DH + 1, 512], f32, name="ps_acc", tag="acc")
                for jc in range(njc):
                    eh, i0, n = es[jc]
                    nc.tensor.matmul(
                        acc[:, i0 : i0 + n],
                        sb_v[:, jc, h * 65 : (h + 1) * 65],
                        eh,
                        start=(jc == 0),
                        stop=(jc == njc - 1),
                    )
                # normalize: reciprocal of the rowsum row, broadcast across
                # partitions on the (otherwise idle) GpSimd engine, then one
                # PSUM-reading multiply straight into attnT
                rsk = rcpool.tile([1, 512], f32, name="rsk", tag="rsk", bufs=2)
                nc.scalar.copy(rsk[:], acc[DH : DH + 1, :])
                rck = rcpool.tile([1, 512], f32, name="rck", tag="rck", bufs=3)
                nc.vector.reciprocal_approx_fast(rck[:], rsk[:])
                bck = rcpool.tile([DH, 512], f32, name="bck", tag="bck", bufs=3)
                nc.gpsimd.partition_broadcast(bck[:], rck[:])
                dst = sb_attnT[po : po + DH, ic * 512 : (ic + 1) * 512]
                nc.vector.tensor_mul(dst, acc[0:DH, :], bck[:])
            # project this query block right away to overlap the output DMA
            # with the remaining attention blocks
            for sc in range(4 * ic, 4 * (ic + 1)):
                emit_proj(sc)


def build(mm_dt=mybir.dt.bfloat16):
    key = str(mm_dt)
    if key in _compiled:
        return _compiled[key]
    nc = bacc.Bacc("TRN2", target_bir_lowering=False, debug=False, num_devices=N_CORES)
    xt = nc.dram_tensor("xt", [D, S], mm_dt, kind="ExternalInput").ap()
    w = nc.dram_tensor("w", [D, 384], mm_dt, kind="ExternalInput").ap()
    wout = nc.dram_tensor("wout", [P, D], mm_dt, kind="ExternalInput").ap()
    maskt = nc.dram_tensor("maskt", [P, 4, 512], mm_dt, kind="ExternalInput").ap()
    ident = nc.dram_tensor("ident", [P, P], mm_dt, kind="ExternalInput").ap()
    out = nc.dram_tensor("out", [S, D], mm_dt, kind="ExternalOutput").ap()
    with tile.TileContext(nc) as tc:
        _emit(nc, tc, mm_dt, xt, w, wout, maskt, ident, out)
    nc.compile()
    _compiled[key] = nc
    return nc


def _np_dt(mm_dt):
    if mm_dt == mybir.dt.bfloat16:
        import ml_dtypes

        return ml_dtypes.bfloat16
    return np.float32


def make_inputs(x, Wqkv, Wout, np_dt):
    """Host-side shard/layout prep -> per-core input maps."""
    x = np.ascontiguousarray(np.asarray(x, np.float32))
    Wqkv = np.asarray(Wqkv, np.float32)
    Wout = np.asarray(Wout, np.float32)
    xT = np.ascontiguousarray(x.T).astype(np_dt)  # [D, S]
    j = np.arange(512, dtype=np.int64)
    m512 = (j[:, None] <= j[None, :]).astype(np.float32)  # [J, i]: J <= i
    mask = np.ascontiguousarray(
        m512.reshape(4, 128, 512).transpose(1, 0, 2)
    ).astype(np_dt)  # [p, r, i] = (128r + p <= i)
    in_maps = []
    for c in range(N_CORES):
        wq = Wqkv[:, 128 * c : 128 * (c + 1)] * (1.0 / np.sqrt(DH))
        wk = Wqkv[:, D + 128 * c : D + 128 * (c + 1)]
        wv = Wqkv[:, 2 * D + 128 * c : 2 * D + 128 * (c + 1)]
        w_loc = np.ascontiguousarray(np.concatenate([wq, wk, wv], axis=1))
        wout_loc = np.ascontiguousarray(Wout[128 * c : 128 * (c + 1), :])
        in_maps.append(
            {
                "xt": xT,
                "w": w_loc.astype(np_dt),
                "wout": wout_loc.astype(np_dt),
                "maskt": mask,
                "ident": np.eye(P, dtype=np_dt),
            }
        )
    return in_maps


def kernel(x, Wqkv, Wout, bias, mm_dt=mybir.dt.bfloat16, **run_kwargs):
    nc = build(mm_dt)
    in_maps = make_inputs(x, Wqkv, Wout, _np_dt(mm_dt))
    res = run_bass_kernel_spmd(nc, in_maps, core_ids=list(range(N_CORES)), **run_kwargs)
    acc = np.zeros((S, D), np.float64)
    for c in range(N_CORES):
        acc += res.results[c]["out"].astype(np.float64)
    acc += np.asarray(bias, np.float64)[None, :]
    return acc.astype(np.float32)


# revision 22
# speedup vs baseline: 1.8699x; 1.0522x over previous
"""Causal self-attention (S=2048, D=1024, 16 heads x 64) on 8 Trainium2 cores.

Tensor-parallel sharding: 2 heads per core. Each core computes
  qkv_local = x @ Wqkv[:, local]      (local q/k/v columns, q pre-scaled 1/8)
  attn_h    = softmax(mask(q_h k_h^T)) v_h          for its 2 heads
  partial   = concat(attn) @ Wout[local_rows, :]    (128 rows of Wout)
and the host sums the 8 bf16 partials (+bias).

All on-chip data is bf16 (PSUM accumulation in fp32); end-to-end rel err
vs the fp32 reference is ~4.4e-3, dominated by the input rounding.

On-chip layout: q^T/k^T are single [128, S] tiles with head-dim on
partitions (h0 rows 0-63, h1 rows 64-127); logit matmuls contract K=64
over the head's partition range. Logits are computed transposed
([key, query]) so exp(logits) blocks feed the probs@v matmul directly as
the moving operand; v carries a ones-column per head so the same
accumulation also produces the softmax row-sums. No max-subtraction:
logits are ~N(0,1) after the 1/8 scale. Masked entries are zeroed after
exp (GpSimd, off the PE/ACT critical path).

Schedule notes: 8 warm-up matmuls on a zero tile run during the initial
DMA wait so the PE_HAM clock gate (1.2 -> 2.4 GHz after ~3.4us of
sustained activity) is already open when real work starts. Attention
emits both heads' logit groups interleaved so one head's PE matmuls hide
the other head's serial exp latency on the scalar engine.
"""

import numpy as np

import concourse.bass as bass
import concourse.mybir as mybir
import concourse.tile as tile
from concourse import bacc
from concourse.bass_utils import run_bass_kernel_spmd

S = 2048
D = 1024
DH = 64
N_CORES = 8

P = 128
NB512 = S // 512  # 512-wide query chunks
NB128 = S // 128  # 128-wide chunks
KO = D // P  # contraction chunks for the projections

F32 = mybir.dt.float32

_compiled = {}


def _emit(nc, tc, mm_dt, xt, w, wout, maskt, ident, out):
    f32 = F32
    with (
        tc.tile_pool(name="const", bufs=1) as const,
        tc.tile_pool(name="epool", bufs=8) as epool,
        tc.tile_pool(name="opool", bufs=6) as opool,
        tc.tile_pool(name="rcpool", bufs=1) as rcpool,
        tc.tile_pool(name="psmm", bufs=2, space="PSUM") as psmm,
        tc.tile_pool(name="psacc", bufs=2, space="PSUM") as psacc,
    ):
        sb_xT = const.tile([P, KO, S], mm_dt, name="sb_xT")
        sb_w = const.tile([P, KO, 384], mm_dt, name="sb_w")
        sb_wout = const.tile([P, D], mm_dt, name="sb_wout")
        sb_mask = const.tile([P, 4, 512], mm_dt, name="sb_mask")
        # per-head q^T/k^T padded to K=128 with zero rows 64-127: K=64
        # matmuls run slow and do not register PE activity for the HAM
        # clock gate (observed: throttle_active jumps 18us -> 80us)
        sb_qT = [const.tile([P, S], mm_dt, name=f"sb_qT{h}") for h in (0, 1)]
        sb_kT = [const.tile([P, S], mm_dt, name=f"sb_kT{h}") for h in (0, 1)]
        sb_v = const.tile([P, NB128, 130], mm_dt, name="sb_v")
        sb_vT = const.tile([P, S], mm_dt, name="sb_vT")
        sb_attnT = const.tile([P, S], mm_dt, name="sb_attnT")
        sb_ident = const.tile([P, P], mm_dt, name="sb_ident")
        sb_warm = const.tile([P, 512], mm_dt, name="sb_warm")

        # PE warm-up: ~3.4us of dependency-free matmuls on a zero tile keep
        # the HAM activity window busy while the first input DMAs stream, so
        # the clock gate is fully open (2.4 GHz) when real matmuls start.
        # Tapered sizes: once the first real operands land, at most one
        # small spam matmul delays the real stream.
        nc.vector.memset(sb_warm[:], 0.0)
        for rows in [512] * 6 + [128] * 16:
            wps = psmm.tile([P, 512], f32, name="ps_warm", tag="mm")
            nc.tensor.matmul(
                wps[:, :rows], sb_warm[:, 0:P], sb_warm[:, :rows],
                start=True, stop=True,
            )

        # loads: small weights on the GpSimd SWDGE queue; xT column blocks
        # si-major so the first q/k chunk only waits on the first ~1MB
        for o in range(KO):
            weng = nc.scalar if o % 2 == 0 else nc.gpsimd
            weng.dma_start(sb_w[:, o, :], w[o * P : (o + 1) * P, :])
        xt3 = xt.rearrange("(o p) s -> p o s", p=P)
        for si in range(NB512):
            sl = slice(si * 512, (si + 1) * 512)
            for oo in range(0, KO, 2):
                nc.sync.dma_start(
                    sb_xT[:, oo : oo + 2, sl], xt3[:, oo : oo + 2, sl]
                )
        nc.gpsimd.dma_start(sb_mask[:], maskt[:])
        nc.gpsimd.dma_start(sb_wout[:], wout[:])
        nc.gpsimd.dma_start(sb_ident[:], ident[:])
        nc.gpsimd.memset(sb_v[:, :, DH], 1.0)
        nc.gpsimd.memset(sb_v[:, :, 129], 1.0)
        for h in (0, 1):
            nc.gpsimd.memset(sb_qT[h][DH:P, :], 0.0)
            nc.gpsimd.memset(sb_kT[h][DH:P, :], 0.0)

        # q^T / k^T producer: [c, s] = sum_D W[D, c] * xT[D, s]. The q and k
        # accumulation chains are interleaved per xT o-chunk so that during
        # the DMA-paced startup each arriving chunk feeds 2 matmuls back to
        # back (the early HAM window stays busy; re-throttle avoided).
        def emit_qk(si):
            ps = {}
            for cc in (0, 1):
                ps[cc] = psmm.tile([P, 512], f32, name="ps_qk", tag="mm")
            for o in range(KO):
                for cc in (0, 1):
                    nc.tensor.matmul(
                        ps[cc][:],
                        sb_w[:, o, cc * P : (cc + 1) * P],
                        sb_xT[:, o, si * 512 : (si + 1) * 512],
                        start=(o == 0),
                        stop=(o == KO - 1),
                        skip_group_check=True,
                    )
            sl = slice(si * 512, (si + 1) * 512)
            for cc, dest in ((0, sb_qT), (1, sb_kT)):
                nc.vector.tensor_copy(dest[0][0:DH, sl], ps[cc][0:DH, :])
                nc.vector.tensor_copy(dest[1][0:DH, sl], ps[cc][DH:P, :])

        # v^T producer (same efficient N=512 shape as q/k), then PE-mode
        # transposes turn each 128x128 block into v natural layout
        def emit_vT(si):
            psv = psmm.tile([P, 512], f32, name="ps_vT", tag="mm")
            for o in range(KO):
                nc.tensor.matmul(
                    psv[:],
                    sb_w[:, o, 256:384],
                    sb_xT[:, o, si * 512 : (si + 1) * 512],
                    start=(o == 0),
                    stop=(o == KO - 1),
                )
            nc.vector.tensor_copy(sb_vT[:, si * 512 : (si + 1) * 512], psv[:])

        def emit_v(sc):
            pt = psmm.tile([P, P], mm_dt, name="ps_t", tag="mm")
            nc.tensor.transpose(
                pt[:], sb_vT[:, sc * P : (sc + 1) * P], sb_ident[:]
            )
            nc.vector.tensor_copy(sb_v[:, sc, 0:DH], pt[:, 0:DH])
            nc.vector.tensor_copy(sb_v[:, sc, DH + 1 : 129], pt[:, DH:P])

        # output projection for one 128-row query chunk; the final block's
        # PSUM->SBUF copies go to the scalar engine (its exp stream is done
        # by then, while the vector engine still carries the normalizes)
        def emit_proj(sc, on_scalar=False):
            for ec in range(D // 512):
                pp = psmm.tile([P, 512], f32, name="ps_p", tag="mm")
                nc.tensor.matmul(
                    pp[:],
                    sb_attnT[:, sc * P : (sc + 1) * P],
                    sb_wout[:, ec * 512 : (ec + 1) * 512],
                    start=True,
                    stop=True,
                )
                ot = opool.tile([P, 512], mm_dt, name="ot", tag="ot")
                if on_scalar:
                    nc.scalar.copy(ot[:], pp[:])
                else:
                    nc.vector.tensor_copy(ot[:], pp[:])
                nc.sync.dma_start(
                    out[sc * P : (sc + 1) * P, ec * 512 : (ec + 1) * 512], ot[:]
                )

        # interleaved schedule: produce q/k/v just-in-time, run causal
        # attention blocks (logits kept transposed [j, i]), project each
        # query chunk as soon as both heads are normalized
        emit_qk(0)
        emit_vT(0)
        for sc in range(4):
            emit_v(sc)
        for ic in range(NB512):
            if ic + 1 < NB512:
                emit_qk(ic + 1)
                emit_vT(ic + 1)
                for sc in range(4 * (ic + 1), 4 * (ic + 2)):
                    emit_v(sc)
            for h in (0, 1):
                po = h * DH
                njc = 4 * (ic + 1)
                # pair logit blocks into shared 2-bank PSUM tiles (one exp
                # covers each pair); diagonal blocks are narrowed to the
                # columns that survive the causal mask: jc=4ic+r only needs
                # query columns i >= 128r within the 512-block
                groups = []  # list of [(jc, col_start, n, i0), ...]
                for jp in range(2 * ic):
                    groups.append(
                        [(2 * jp, 0, 512, 0), (2 * jp + 1, 512, 512, 0)]
                    )
                groups.append(
                    [(4 * ic, 0, 512, 0), (4 * ic + 1, 512, 384, 128)]
                )
                groups.append(
                    [(4 * ic + 2, 0, 256, 256), (4 * ic + 3, 256, 256, 256)]
                )
                es = {}
                for grp in groups:
                    tot = grp[-1][1] + grp[-1][2]
                    pl = psmm.tile([P, 1024], f32, name="ps_l", tag="mm2")
                    for jc, cs, n, i0 in grp:
                        nc.tensor.matmul(
                            pl[:, cs : cs + n],
                            sb_kT[h][:, jc * P : (jc + 1) * P],
                            sb_qT[h][:, ic * 512 + i0 : ic * 512 + i0 + n],
                            start=True,
                            stop=True,
                        )
                    e = epool.tile([P, 1024], mm_dt, name="e_t", tag="e")
                    nc.scalar.activation(
                        e[:, :tot], pl[:, :tot], mybir.ActivationFunctionType.Exp
                    )
                    for jc, cs, n, i0 in grp:
                        r = jc - 4 * ic
                        eh = e[:, cs : cs + n]
                        if r >= 0:
                            nc.vector.tensor_mul(
                                eh, eh, sb_mask[:, r, i0 : i0 + n]
                            )
                        es[jc] = (eh, i0, n)
                acc = psacc.tile([DH + 1, 512], f32, name="ps_acc", tag="acc")
                for jc in range(njc):
                    eh, i0, n = es[jc]
                    nc.tensor.matmul(
                        acc[:, i0 : i0 + n],
                        sb_v[:, jc, h * 65 : (h + 1) * 65],
                        eh,
                        start=(jc == 0),
                        stop=(jc == njc - 1),
                    )
                # normalize: reciprocal of the rowsum row, broadcast across
                # partitions on the (otherwise idle) GpSimd engine, then one
                # PSUM-reading multiply straight into attnT
                rsk = rcpool.tile([1, 512], f32, name="rsk", tag="rsk", bufs=2)
                nc.scalar.copy(rsk[:], acc[DH : DH + 1, :])
                rck = rcpool.tile([1, 512], f32, name="rck", tag="rck", bufs=3)
                nc.vector.reciprocal_approx_fast(rck[:], rsk[:])
                bck = rcpool.tile([DH, 512], f32, name="bck", tag="bck", bufs=3)
                nc.gpsimd.partition_broadcast(bck[:], rck[:])
                dst = sb_attnT[po : po + DH, ic * 512 : (ic + 1) * 512]
                nc.vector.tensor_mul(dst, acc[0:DH, :], bck[:])
            # project this query block right away to overlap the output DMA
            # with the remaining attention blocks
            for sc in range(4 * ic, 4 * (ic + 1)):
                emit_proj(sc, on_scalar=(ic == NB512 - 1))


def build(mm_dt=mybir.dt.bfloat16):
    key = str(mm_dt)
    if key in _compiled:
        return _compiled[key]
    nc = bacc.Bacc("TRN2", target_bir_lowering=False, debug=False, num_devices=N_CORES)
    xt = nc.dram_tensor("xt", [D, S], mm_dt, kind="ExternalInput").ap()
    w = nc.dram_tensor("w", [D, 384], mm_dt, kind="ExternalInput").ap()
    wout = nc.dram_tensor("wout", [P, D], mm_dt, kind="ExternalInput").ap()
    maskt = nc.dram_tensor("maskt", [P, 4, 512], mm_dt, kind="ExternalInput").ap()
    ident = nc.dram_tensor("ident", [P, P], mm_dt, kind="ExternalInput").ap()
    out = nc.dram_tensor("out", [S, D], mm_dt, kind="ExternalOutput").ap()
    with tile.TileContext(nc) as tc:
        _emit(nc, tc, mm_dt, xt, w, wout, maskt, ident, out)
    nc.compile()
    _compiled[key] = nc
    return nc


def _np_dt(mm_dt):
    if mm_dt == mybir.dt.bfloat16:
        import ml_dtypes

        return ml_dtypes.bfloat16
    return np.float32


def make_inputs(x, Wqkv, Wout, np_dt):
    """Host-side shard/layout prep -> per-core input maps."""
    x = np.ascontiguousarray(np.asarray(x, np.float32))
    Wqkv = np.asarray(Wqkv, np.float32)
    Wout = np.asarray(Wout, np.float32)
    xT = np.ascontiguousarray(x.T).astype(np_dt)  # [D, S]
    j = np.arange(512, dtype=np.int64)
    m512 = (j[:, None] <= j[None, :]).astype(np.float32)  # [J, i]: J <= i
    mask = np.ascontiguousarray(
        m512.reshape(4, 128, 512).transpose(1, 0, 2)
    ).astype(np_dt)  # [p, r, i] = (128r + p <= i)
    in_maps = []
    for c in range(N_CORES):
        wq = Wqkv[:, 128 * c : 128 * (c + 1)] * (1.0 / np.sqrt(DH))
        wk = Wqkv[:, D + 128 * c : D + 128 * (c + 1)]
        wv = Wqkv[:, 2 * D + 128 * c : 2 * D + 128 * (c + 1)]
        w_loc = np.ascontiguousarray(np.concatenate([wq, wk, wv], axis=1))
        wout_loc = np.ascontiguousarray(Wout[128 * c : 128 * (c + 1), :])
        in_maps.append(
            {
                "xt": xT,
                "w": w_loc.astype(np_dt),
                "wout": wout_loc.astype(np_dt),
                "maskt": mask,
                "ident": np.eye(P, dtype=np_dt),
            }
        )
    return in_maps


def kernel(x, Wqkv, Wout, bias, mm_dt=mybir.dt.bfloat16, **run_kwargs):
    nc = build(mm_dt)
    in_maps = make_inputs(x, Wqkv, Wout, _np_dt(mm_dt))
    res = run_bass_kernel_spmd(nc, in_maps, core_ids=list(range(N_CORES)), **run_kwargs)
    acc = np.zeros((S, D), np.float64)
    for c in range(N_CORES):
        acc += res.results[c]["out"].astype(np.float64)
    acc += np.asarray(bias, np.float64)[None, :]
    return acc.astype(np.float32)
